# revision 17
# baseline (speedup 1.0000x reference)
"""ChebyKAN Trainium2 kernel.

Reference computation:
    t = tanh(x)                      # x: [8192, 768]
    cheby[b,i,d] = T_d(t[b,i])       # Chebyshev polys, d = 0..8
    out[b,j] = sum_{i,d} cheby[b,i,d] * coefficients[i,j,d]

Strategy (data-parallel over batch across 8 cores):
  - Each core gets a 1024-row batch shard, transposed on host to xt [768, 1024]
    so the contraction dim (in_features) lands on SBUF partitions.
  - out.T[j, b] = sum_k coeffK[k, j] * chebyK[k, b], K = 6*128 i-tiles x 8
    degrees (d=0 contributes a j-constant folded into a bias at PSUM drain).
  - Mixed precision on the PE: degrees 1-6 run as bf16 matmuls (216 ns per
    N=512 issue, HW-measured); degrees 7-8 are packed into ONE fp8e4
    DoubleRow matmul per j-tile (2 fp8 weights/cell, K=256 per issue,
    ~1.44x bf16 throughput).  Measured end-to-end l2 error 1.5e-2 vs the
    2e-2 budget; fp8 products are exact in HW (e6m3 upcast, e10m10
    product, f32 accumulate).  All coefficients are pre-scaled by
    S = 6912 so the d7/d8 weights land in e4m3's normal range; the PSUM
    drain rescales by 1/S (fused into the activation's input scale).
  - fp32 PSUM accum.  The whole coefficient array is SBUF-resident
    (~8.3 MB), DMA'd once in the prologue as per-degree 196KB slices
    (a single big descriptor only moves ~125 GB/s; slices pipeline).
    Both batch-half passes reuse it; the Sync queue goes quiet after the
    prologue (it only issues DMA descriptors at ~1.6/us).
  - Two passes over batch halves of 512: per pass, all 6 j-tiles
    accumulate in 6 single-bank PSUM tiles over the 42 issues/pass.
  - Chebyshev tiles via product identities: T2=2t^2-1, T3=2tT2-t, T4=2T2^2-1,
    T5=2T2T3-t, T6=2T3^2-1, T7=2T3T4-t, T8=2T4^2-1.  The generator chain
    (t, T2, T3, T4) is kept in f32 — a low-precision recurrence amplifies
    the t-rounding by |T_d'| ~ d^2 near |t|=1 — and each T_d is rounded
    once for the matmul rhs.  No GpSimd compute: concurrent GpSimd
    elementwise steals DVE ports (~3x DVE slowdown, HW-measured).
  - Deep software pipeline: block b's whole cheby chain is emitted one
    block ahead of its matmuls; the tanh pair carries a sim-time floor
    (tile_wait_until) so the list scheduler cannot hoist it ahead of the
    previous chain on the in-order scalar queue and head-of-line block on
    its xt DMA (the scheduler's DMA model is optimistic).
  - The last block of EACH pass runs jt-major with inline staggered PSUM
    drains, so the next pass finds recycled banks without stalling.
  - A few dummy matmuls bridge the PE from warm-tile-ready to the first
    real matmul so the HAM throttle window is burnt before real work.
  - Output is written bf16 (halves the drain DMA); host upcasts.
"""

import sys

for _p in ("/opt/trn_rl_repo",):
    if _p not in sys.path:
        sys.path.insert(0, _p)

import ml_dtypes
import numpy as np

import concourse.bass as bass
import concourse.mybir as mybir
import concourse.tile as tile
from concourse import bacc
from concourse import bass_utils
from concourse.tile import TileContext

F32 = mybir.dt.float32
BF16 = mybir.dt.bfloat16
F8E4 = mybir.dt.float8e4
AF = mybir.ActivationFunctionType
OP = mybir.AluOpType
DR = mybir.MatmulPerfMode.DoubleRow

B, I, J, D1 = 8192, 768, 768, 9  # batch, in_features, out_features, degree+1
NCORES = 8
BPC = B // NCORES      # 1024 batch rows per core
IT = I // 128          # 6 i-tiles
JT = J // 128          # 6 j-tiles
HB = 512               # half-batch (matmul N)
NDUMMY = 5             # PE warm-up matmuls
SCALE = float(I * D1)  # coefficient pre-scale (6912)

_CACHE = {}


def _build_nc():
    nc = bacc.Bacc("TRN2", target_bir_lowering=False, debug=False,
                   num_devices=NCORES)
    xt = nc.dram_tensor("xt", [I, BPC], F32, kind="ExternalInput").ap()
    # coeff[it, row, dm1*J + j] = S * coefficients[it*128+row, j, dm1+1]
    coeff = nc.dram_tensor("coeff", [IT, 128, 6 * J], BF16,
                           kind="ExternalInput").ap()
    # coeff8[it, row, kd, j] = S * coefficients[it*128+row, j, 7+kd]
    coeff8 = nc.dram_tensor("coeff8", [IT, 128, 2, J], F8E4,
                            kind="ExternalInput").ap()
    bias = nc.dram_tensor("bias", [128, JT], F32, kind="ExternalInput").ap()
    out = nc.dram_tensor("out", [J, BPC], BF16, kind="ExternalOutput").ap()

    blocks = [(half, it) for half in range(2) for it in range(IT)]
    NB = len(blocks)
    INV_S = 1.0 / SCALE

    with TileContext(nc) as tc:
        with (
            tc.tile_pool(name="xtp", bufs=1) as xt_pool,
            tc.tile_pool(name="work", bufs=3) as work,
            tc.tile_pool(name="tanp", bufs=4) as tan_pool,
            tc.tile_pool(name="coeffp", bufs=1) as coeff_pool,
            tc.tile_pool(name="outp", bufs=6) as out_pool,
            tc.tile_pool(name="biasp", bufs=1) as bias_pool,
            tc.tile_pool(name="psum", bufs=8, space="PSUM") as psum_pool,
        ):
            # PE warm-up scratch (zeroed; HAM un-throttles after ~3.4us of
            # sustained matmul activity; the real stream takes over while
            # still inside the cold window).
            warm_f = work.tile([128, HB], F32, name="warm_f", tag="warm_f",
                               bufs=1)
            nc.vector.memset(warm_f, 0.0)
            warm = work.tile([128, HB], BF16, name="warm", tag="warm", bufs=1)
            nc.vector.tensor_copy(warm, warm_f)

            bias_all = bias_pool.tile([128, JT], F32, name="bias_all",
                                      tag="bias_all")

            xt_tiles = [
                xt_pool.tile([128, BPC], F32, name=f"xtt{it}", tag=f"xtt{it}")
                for it in range(IT)
            ]
            ct_tiles = [
                coeff_pool.tile([128, 6 * J], BF16, name=f"ct{it}",
                                tag=f"ct{it}")
                for it in range(IT)
            ]
            w78_tiles = [
                coeff_pool.tile([128, 2, J], F8E4, name=f"w78_{it}",
                                tag=f"w78_{it}")
                for it in range(IT)
            ]

            def issue_tanh(bi, wait_ms=None):
                half, it = blocks[bi]
                hs = slice(half * HB, (half + 1) * HB)
                t_b = tan_pool.tile([128, HB], BF16, name="t_b", tag="t_b")
                t = tan_pool.tile([128, HB], F32, name="t", tag="t")
                if wait_ms is None:
                    nc.scalar.activation(t_b, xt_tiles[it][:, hs], AF.Tanh)
                    nc.scalar.activation(t, xt_tiles[it][:, hs], AF.Tanh)
                else:
                    # Scheduling hint: the list scheduler orders the in-order
                    # scalar queue by sim-readiness, and its DMA model is
                    # optimistic — an un-hinted pipelined tanh gets hoisted
                    # ahead of the previous block's chain ops and then
                    # head-of-line blocks them on its xt DMA.
                    with tc.tile_wait_until(wait_ms):
                        nc.scalar.activation(t_b, xt_tiles[it][:, hs],
                                             AF.Tanh)
                        nc.scalar.activation(t, xt_tiles[it][:, hs], AF.Tanh)
                return t, t_b

            def emit_chain(t, t_b):
                """Scalar+vector ops producing the cheby rhs tiles:
                bf16 t..T6 plus the fp8 DoubleRow-packed (T7, T8) pair."""
                T78 = work.tile([128, 2, HB], F8E4, name="T78", tag="T78")
                # T2 = 2 t^2 - 1
                sq = work.tile([128, HB], F32, name="sq", tag="sq")
                nc.scalar.activation(sq, t, AF.Square)
                T2 = work.tile([128, HB], F32, name="T2", tag="T2")
                nc.vector.tensor_scalar(T2, sq, 2.0, 1.0, OP.mult,
                                        OP.subtract)
                T2_b = work.tile([128, HB], BF16, name="T2_b", tag="T2_b")
                nc.vector.tensor_copy(T2_b, T2)
                # T3 = 2 t T2 - t
                P = work.tile([128, HB], F32, name="P", tag="P")
                nc.vector.tensor_mul(P, t, T2)
                T3 = work.tile([128, HB], F32, name="T3", tag="T3")
                nc.vector.scalar_tensor_tensor(T3, P, 2.0, t, OP.mult,
                                               OP.subtract)
                T3_b = work.tile([128, HB], BF16, name="T3_b", tag="T3_b")
                nc.scalar.activation(T3_b, T3, AF.Identity)
                # T4 = 2 T2^2 - 1
                sq = work.tile([128, HB], F32, name="sq", tag="sq")
                nc.scalar.activation(sq, T2, AF.Square)
                T4 = work.tile([128, HB], F32, name="T4", tag="T4")
                nc.vector.tensor_scalar(T4, sq, 2.0, 1.0, OP.mult,
                                        OP.subtract)
                T4_b = work.tile([128, HB], BF16, name="T4_b", tag="T4_b")
                nc.vector.tensor_copy(T4_b, T4)
                # T5 = 2 T2 T3 - t
                P = work.tile([128, HB], F32, name="P", tag="P")
                nc.vector.tensor_mul(P, T2, T3)
                T5_b = work.tile([128, HB], BF16, name="T5_b", tag="T5_b")
                nc.vector.scalar_tensor_tensor(T5_b, P, 2.0, t, OP.mult,
                                               OP.subtract)
                # T6 = 2 T3^2 - 1
                sq = work.tile([128, HB], F32, name="sq", tag="sq")
                nc.scalar.activation(sq, T3, AF.Square)
                T6_b = work.tile([128, HB], BF16, name="T6_b", tag="T6_b")
                nc.vector.tensor_scalar(T6_b, sq, 2.0, 1.0, OP.mult,
                                        OP.subtract)
                # T7 = 2 T3 T4 - t  (fp8, DoubleRow k-subtile 0)
                T78s = work.tile([128, 2, HB], F8E4, name="T78s", tag="T78s")
                P = work.tile([128, HB], F32, name="P", tag="P")
                nc.vector.tensor_mul(P, T3, T4)
                nc.vector.scalar_tensor_tensor(T78s[:, 0, :], P, 2.0, t,
                                               OP.mult, OP.subtract)
                # T8 = 2 T4^2 - 1  (fp8, DoubleRow k-subtile 1)
                sq = work.tile([128, HB], F32, name="sq", tag="sq")
                nc.scalar.activation(sq, T4, AF.Square)
                nc.vector.tensor_scalar(T78s[:, 1, :], sq, 2.0, 1.0, OP.mult,
                                        OP.subtract)
                # Repack through one full-tile copy: the DVE queue is
                # in-order so this read can't race the two slice writes
                # above, and the DoubleRow matmul then depends on a single
                # whole-tile writer (slice-write dep tracking raced ~1-in-4
                # runs, dropping one block's d7/d8 contribution).
                nc.vector.tensor_copy(T78, T78s)
                return (t_b, T2_b, T3_b, T4_b, T5_b, T6_b), T78

            # Prologue DMA order on the Sync queue == rough transfer/land
            # order: first tanh needs xt0's first half, the first matmuls
            # need coeff it=0 degree by degree; everything else follows at
            # the rate the blocks consume it.
            def ct_slice(it, dm1):
                nc.sync.dma_start(ct_tiles[it][:, dm1 * J:(dm1 + 1) * J],
                                  coeff[it][:, dm1 * J:(dm1 + 1) * J])

            def ct_f8(it):
                nc.sync.dma_start(w78_tiles[it], coeff8[it])

            def ct_all(it):
                for dm1 in range(6):
                    ct_slice(it, dm1)
                ct_f8(it)

            def xt_dma(it, half=None):
                if half is None:
                    nc.sync.dma_start(xt_tiles[it],
                                      xt[it * 128:(it + 1) * 128, :])
                else:
                    hs = slice(half * HB, (half + 1) * HB)
                    nc.sync.dma_start(xt_tiles[it][:, hs],
                                      xt[it * 128:(it + 1) * 128, hs])

            xt_dma(0, 0)
            ct_slice(0, 0)
            ct_slice(0, 1)
            xt_dma(1)
            ct_slice(0, 2)
            ct_slice(0, 3)
            xt_dma(2)
            ct_slice(0, 4)
            ct_slice(0, 5)
            ct_f8(0)
            ct_all(1)
            xt_dma(0, 1)
            nc.sync.dma_start(bias_all, bias)
            xt_dma(3)
            ct_all(2)
            xt_dma(4)
            ct_all(3)
            xt_dma(5)
            ct_all(4)
            ct_all(5)

            # Software-pipeline prologue: tanh(0) + chain(0).
            chain_pipe = [None] * NB
            chain_pipe[0] = emit_chain(*issue_tanh(0))

            def drain(jt, ps_t, hs, use_scalar, dma_engine):
                ob = out_pool.tile([128, HB], BF16, name="ob", tag="ob")
                if use_scalar:
                    nc.scalar.activation(ob, ps_t, AF.Identity,
                                         bias=bias_all[:, jt:jt + 1],
                                         scale=INV_S)
                else:
                    obf = out_pool.tile([128, HB], F32, name="obf",
                                        tag="obf")
                    nc.vector.tensor_scalar(obf, ps_t, INV_S, 0.0, OP.mult,
                                            OP.add)
                    nc.vector.tensor_scalar_add(ob, obf,
                                                bias_all[:, jt:jt + 1])
                dma_engine.dma_start(out[jt * 128:(jt + 1) * 128, hs], ob)

            ps = None
            for bi, (half, it) in enumerate(blocks):
                hs = slice(half * HB, (half + 1) * HB)
                ctt = ct_tiles[it]
                w78 = w78_tiles[it]
                if it == 0:
                    ps = [psum_pool.tile([128, HB], F32, name="ps", tag="ps")
                          for _ in range(JT)]
                if bi == 0:
                    # dummy matmuls bridge PE from warm-tile-ready to the
                    # first real matmul; overwritten by the real first matmul
                    # (start=True clears has_written)
                    for _ in range(NDUMMY):
                        nc.tensor.matmul(ps[0], lhsT=warm[:, :128], rhs=warm,
                                         start=True, stop=True)

                Ts, T78 = chain_pipe[bi]
                first = it == 0
                last = it == IT - 1
                if last:
                    # Last block of each pass: jt-major so each j-tile's
                    # accumulation finishes staggered and its PSUM drain
                    # (copy + store) pipelines behind the remaining matmuls;
                    # the next pass finds recycled banks without stalling.
                    for jt in range(JT):
                        for dm1, Td in enumerate(Ts):
                            nc.tensor.matmul(
                                ps[jt],
                                lhsT=ctt[:, dm1 * J + jt * 128:
                                         dm1 * J + (jt + 1) * 128],
                                rhs=Td,
                                start=False,
                                stop=False,
                            )
                        nc.tensor.matmul(
                            ps[jt],
                            lhsT=w78[:, :, jt * 128:(jt + 1) * 128],
                            rhs=T78,
                            perf_mode=DR,
                            start=False,
                            stop=True,
                        )
                        drain(jt, ps[jt], hs, use_scalar=(jt % 2 == 1),
                              dma_engine=(nc.scalar if jt % 2 == 1
                                          else (nc.sync if half else
                                                nc.gpsimd)))
                else:
                    for dm1, Td in enumerate(Ts):
                        for jt in range(JT):
                            nc.tensor.matmul(
                                ps[jt],
                                lhsT=ctt[:, dm1 * J + jt * 128:
                                         dm1 * J + (jt + 1) * 128],
                                rhs=Td,
                                start=(first and dm1 == 0),
                                stop=False,
                            )
                    for jt in range(JT):
                        nc.tensor.matmul(
                            ps[jt],
                            lhsT=w78[:, :, jt * 128:(jt + 1) * 128],
                            rhs=T78,
                            perf_mode=DR,
                            start=False,
                            stop=False,
                        )

                # Produce block bi+1's chain AFTER block bi's matmuls and
                # drains: emission order is the scheduler's priority
                # tie-break, and at pass ends the PSUM drains must jump the
                # in-order scalar/vector queues ahead of next-block chain
                # ops, or the next pass stalls on bank recycling.
                if bi + 1 < NB:
                    chain_pipe[bi + 1] = emit_chain(
                        *issue_tanh(bi + 1, wait_ms=(12.0 + 9.5 * bi) / 1000))

    nc.compile()
    return nc


def _get_nc():
    if "nc" not in _CACHE:
        _CACHE["nc"] = _build_nc()
    return _CACHE["nc"]


def _prep_inputs(x, coefficients):
    x = np.asarray(x, dtype=np.float32)
    coefficients = np.asarray(coefficients, dtype=np.float32)
    xt_full = np.ascontiguousarray(x.T)  # [768, 8192]

    cr = coefficients.reshape(IT, 128, J, D1) * SCALE
    # coeff[it, row, dm1*J + j] = S * C[it*128+row, j, dm1+1], d = 1..6
    arr6 = np.transpose(cr[:, :, :, 1:7], (0, 1, 3, 2))  # [6, 128, 6, 768]
    coeff_in = np.ascontiguousarray(
        arr6.reshape(IT, 128, 6 * J).astype(ml_dtypes.bfloat16))
    # coeff8[it, row, kd, j] = S * C[it*128+row, j, 7+kd]  (TRN e4m3 caps
    # at +-240; values beyond that are ~impossible for N(0,1) but clip to
    # be safe)
    arr78 = np.transpose(cr[:, :, :, 7:9], (0, 1, 3, 2))  # [6, 128, 2, 768]
    coeff8_in = np.ascontiguousarray(
        np.clip(arr78, -240.0, 240.0).astype(ml_dtypes.float8_e4m3))

    bias_in = np.ascontiguousarray(
        coefficients[:, :, 0].sum(axis=0).astype(np.float32).reshape(JT, 128).T
    )

    in_maps = []
    for c in range(NCORES):
        xt_c = np.ascontiguousarray(xt_full[:, c * BPC:(c + 1) * BPC])
        in_maps.append({"xt": xt_c, "coeff": coeff_in, "coeff8": coeff8_in,
                        "bias": bias_in})
    return in_maps


def _run(x, coefficients, trace=False, **run_kwargs):
    nc = _get_nc()
    in_maps = _prep_inputs(x, coefficients)
    res = bass_utils.run_bass_kernel_spmd(
        nc, in_maps, core_ids=list(range(NCORES)), trace=trace, **run_kwargs
    )
    out_full = np.empty((B, J), dtype=np.float32)
    for c in range(NCORES):
        out_full[c * BPC:(c + 1) * BPC, :] = \
            res.results[c]["out"].T.astype(np.float32)
    return out_full, res


def kernel(x, coefficients):
    out, _ = _run(x, coefficients, trace=False)
    return out


if __name__ == "__main__":
    rng = np.random.default_rng(0)
    x = rng.standard_normal((B, I), dtype=np.float32)
    std = 1.0 / (I * D1)
    coefficients = (std * rng.standard_normal((I, J, D1))).astype(np.float32)
    out = kernel(x, coefficients)
    print("out", out.shape, out.dtype, float(np.abs(out).mean()))


# revision 21
# speedup vs baseline: 1.0074x; 1.0074x over previous
"""ChebyKAN Trainium2 kernel.

Reference computation:
    t = tanh(x)                      # x: [8192, 768]
    cheby[b,i,d] = T_d(t[b,i])       # Chebyshev polys, d = 0..8
    out[b,j] = sum_{i,d} cheby[b,i,d] * coefficients[i,j,d]

Strategy (data-parallel over batch across 8 cores):
  - Each core gets a 1024-row batch shard, transposed on host to xt [768, 1024]
    so the contraction dim (in_features) lands on SBUF partitions.
  - out.T[j, b] = sum_k coeffK[k, j] * chebyK[k, b], K = 6*128 i-tiles x 8
    degrees (d=0 contributes a j-constant folded into a bias at PSUM drain).
  - Mixed precision on the PE: degrees 1-6 run as bf16 matmuls (216 ns per
    N=512 issue, HW-measured); degrees 7-8 are packed into ONE fp8e4
    DoubleRow matmul per j-tile (2 fp8 weights/cell, K=256 per issue,
    ~1.44x bf16 throughput).  Measured end-to-end l2 error 1.5e-2 vs the
    2e-2 budget; fp8 products are exact in HW (e6m3 upcast, e10m10
    product, f32 accumulate).  All coefficients are pre-scaled by
    S = 6912 so the d7/d8 weights land in e4m3's normal range; the PSUM
    drain rescales by 1/S (fused into the activation's input scale).
  - fp32 PSUM accum.  The whole coefficient array is SBUF-resident
    (~8.3 MB), DMA'd once in the prologue as per-degree 196KB slices
    (a single big descriptor only moves ~125 GB/s; slices pipeline).
    Both batch-half passes reuse it; the Sync queue goes quiet after the
    prologue (it only issues DMA descriptors at ~1.6/us).
  - Two passes over batch halves of 512: per pass, all 6 j-tiles
    accumulate in 6 single-bank PSUM tiles over the 42 issues/pass.
  - Chebyshev tiles via product identities: T2=2t^2-1, T3=2tT2-t, T4=2T2^2-1,
    T5=2T2T3-t, T6=2T3^2-1, T7=2T3T4-t, T8=2T4^2-1.  The generator chain
    (t, T2, T3, T4) is kept in f32 — a low-precision recurrence amplifies
    the t-rounding by |T_d'| ~ d^2 near |t|=1 — and each T_d is rounded
    once for the matmul rhs.  No GpSimd compute: concurrent GpSimd
    elementwise steals DVE ports (~3x DVE slowdown, HW-measured).
  - Deep software pipeline: block b's whole cheby chain is emitted one
    block ahead of its matmuls; the tanh pair carries a sim-time floor
    (tile_wait_until) so the list scheduler cannot hoist it ahead of the
    previous chain on the in-order scalar queue and head-of-line block on
    its xt DMA (the scheduler's DMA model is optimistic).
  - The last block of EACH pass runs jt-major with inline staggered PSUM
    drains, so the next pass finds recycled banks without stalling.
  - A few dummy matmuls bridge the PE from warm-tile-ready to the first
    real matmul so the HAM throttle window is burnt before real work.
  - Output is written bf16 (halves the drain DMA); host upcasts.
"""

import sys

for _p in ("/opt/trn_rl_repo",):
    if _p not in sys.path:
        sys.path.insert(0, _p)

import ml_dtypes
import numpy as np

import concourse.bass as bass
import concourse.mybir as mybir
import concourse.tile as tile
from concourse import bacc
from concourse import bass_utils
from concourse.tile import TileContext

F32 = mybir.dt.float32
BF16 = mybir.dt.bfloat16
F8E4 = mybir.dt.float8e4
AF = mybir.ActivationFunctionType
OP = mybir.AluOpType
DR = mybir.MatmulPerfMode.DoubleRow

B, I, J, D1 = 8192, 768, 768, 9  # batch, in_features, out_features, degree+1
NCORES = 8
BPC = B // NCORES      # 1024 batch rows per core
IT = I // 128          # 6 i-tiles
JT = J // 128          # 6 j-tiles
HB = 512               # half-batch (matmul N)
NDUMMY = 5             # PE warm-up matmuls
SCALE = float(I * D1)  # coefficient pre-scale (6912)

_CACHE = {}


def _build_nc():
    nc = bacc.Bacc("TRN2", target_bir_lowering=False, debug=False,
                   num_devices=NCORES)
    xt = nc.dram_tensor("xt", [I, BPC], F32, kind="ExternalInput").ap()
    # coeff[it, row, dm1*J + j] = S * coefficients[it*128+row, j, dm1+1]
    coeff = nc.dram_tensor("coeff", [IT, 128, 6 * J], BF16,
                           kind="ExternalInput").ap()
    # coeff8[it, row, kd, j] = S * coefficients[it*128+row, j, 7+kd]
    coeff8 = nc.dram_tensor("coeff8", [IT, 128, 2, J], F8E4,
                            kind="ExternalInput").ap()
    bias = nc.dram_tensor("bias", [128, JT], F32, kind="ExternalInput").ap()
    out = nc.dram_tensor("out", [J, BPC], BF16, kind="ExternalOutput").ap()

    blocks = [(half, it) for half in range(2) for it in range(IT)]
    NB = len(blocks)
    INV_S = 1.0 / SCALE

    with TileContext(nc) as tc:
        with (
            tc.tile_pool(name="xtp", bufs=1) as xt_pool,
            tc.tile_pool(name="work", bufs=3) as work,
            tc.tile_pool(name="tanp", bufs=4) as tan_pool,
            tc.tile_pool(name="coeffp", bufs=1) as coeff_pool,
            tc.tile_pool(name="outp", bufs=6) as out_pool,
            tc.tile_pool(name="biasp", bufs=1) as bias_pool,
            tc.tile_pool(name="psum", bufs=8, space="PSUM") as psum_pool,
        ):
            # PE warm-up scratch (zeroed; HAM un-throttles after ~3.4us of
            # sustained matmul activity; the real stream takes over while
            # still inside the cold window).
            warm_f = work.tile([128, HB], F32, name="warm_f", tag="warm_f",
                               bufs=1)
            nc.vector.memset(warm_f, 0.0)
            warm = work.tile([128, HB], BF16, name="warm", tag="warm", bufs=1)
            nc.vector.tensor_copy(warm, warm_f)

            bias_all = bias_pool.tile([128, JT], F32, name="bias_all",
                                      tag="bias_all")

            xt_tiles = [
                xt_pool.tile([128, BPC], F32, name=f"xtt{it}", tag=f"xtt{it}")
                for it in range(IT)
            ]
            ct_tiles = [
                coeff_pool.tile([128, 6 * J], BF16, name=f"ct{it}",
                                tag=f"ct{it}")
                for it in range(IT)
            ]
            w78_tiles = [
                coeff_pool.tile([128, 2, J], F8E4, name=f"w78_{it}",
                                tag=f"w78_{it}")
                for it in range(IT)
            ]

            def issue_tanh(bi, wait_ms=None):
                half, it = blocks[bi]
                hs = slice(half * HB, (half + 1) * HB)
                t_b = tan_pool.tile([128, HB], BF16, name="t_b", tag="t_b")
                t = tan_pool.tile([128, HB], F32, name="t", tag="t")
                if wait_ms is None:
                    nc.scalar.activation(t_b, xt_tiles[it][:, hs], AF.Tanh)
                    nc.scalar.activation(t, xt_tiles[it][:, hs], AF.Tanh)
                else:
                    # Scheduling hint: the list scheduler orders the in-order
                    # scalar queue by sim-readiness, and its DMA model is
                    # optimistic — an un-hinted pipelined tanh gets hoisted
                    # ahead of the previous block's chain ops and then
                    # head-of-line blocks them on its xt DMA.
                    with tc.tile_wait_until(wait_ms):
                        nc.scalar.activation(t_b, xt_tiles[it][:, hs],
                                             AF.Tanh)
                        nc.scalar.activation(t, xt_tiles[it][:, hs], AF.Tanh)
                return t, t_b

            def emit_chain(t, t_b):
                """Scalar+vector ops producing the cheby rhs tiles:
                bf16 t..T6 plus the fp8 DoubleRow-packed (T7, T8) pair."""
                T78 = work.tile([128, 2, HB], F8E4, name="T78", tag="T78")
                # T2 = 2 t^2 - 1
                sq = work.tile([128, HB], F32, name="sq", tag="sq")
                nc.scalar.activation(sq, t, AF.Square)
                T2 = work.tile([128, HB], F32, name="T2", tag="T2")
                nc.vector.tensor_scalar(T2, sq, 2.0, 1.0, OP.mult,
                                        OP.subtract)
                T2_b = work.tile([128, HB], BF16, name="T2_b", tag="T2_b")
                nc.vector.tensor_copy(T2_b, T2)
                # T3 = 2 t T2 - t
                P = work.tile([128, HB], F32, name="P", tag="P")
                nc.vector.tensor_mul(P, t, T2)
                T3 = work.tile([128, HB], F32, name="T3", tag="T3")
                nc.vector.scalar_tensor_tensor(T3, P, 2.0, t, OP.mult,
                                               OP.subtract)
                T3_b = work.tile([128, HB], BF16, name="T3_b", tag="T3_b")
                nc.scalar.activation(T3_b, T3, AF.Identity)
                # T4 = 2 T2^2 - 1
                sq = work.tile([128, HB], F32, name="sq", tag="sq")
                nc.scalar.activation(sq, T2, AF.Square)
                T4 = work.tile([128, HB], F32, name="T4", tag="T4")
                nc.vector.tensor_scalar(T4, sq, 2.0, 1.0, OP.mult,
                                        OP.subtract)
                T4_b = work.tile([128, HB], BF16, name="T4_b", tag="T4_b")
                nc.vector.tensor_copy(T4_b, T4)
                # T5 = 2 T2 T3 - t
                P = work.tile([128, HB], F32, name="P", tag="P")
                nc.vector.tensor_mul(P, T2, T3)
                T5_b = work.tile([128, HB], BF16, name="T5_b", tag="T5_b")
                nc.vector.scalar_tensor_tensor(T5_b, P, 2.0, t, OP.mult,
                                               OP.subtract)
                # T6 = 2 T3^2 - 1
                sq = work.tile([128, HB], F32, name="sq", tag="sq")
                nc.scalar.activation(sq, T3, AF.Square)
                T6_b = work.tile([128, HB], BF16, name="T6_b", tag="T6_b")
                nc.vector.tensor_scalar(T6_b, sq, 2.0, 1.0, OP.mult,
                                        OP.subtract)
                # T7 = 2 T3 T4 - t  (fp8, DoubleRow k-subtile 0)
                T78s = work.tile([128, 2, HB], F8E4, name="T78s", tag="T78s")
                P = work.tile([128, HB], F32, name="P", tag="P")
                nc.vector.tensor_mul(P, T3, T4)
                nc.vector.scalar_tensor_tensor(T78s[:, 0, :], P, 2.0, t,
                                               OP.mult, OP.subtract)
                # T8 = 2 T4^2 - 1  (fp8, DoubleRow k-subtile 1)
                sq = work.tile([128, HB], F32, name="sq", tag="sq")
                nc.scalar.activation(sq, T4, AF.Square)
                nc.vector.tensor_scalar(T78s[:, 1, :], sq, 2.0, 1.0, OP.mult,
                                        OP.subtract)
                # Repack through one full-tile copy: the DVE queue is
                # in-order so this read can't race the two slice writes
                # above, and the DoubleRow matmul then depends on a single
                # whole-tile writer (slice-write dep tracking raced ~1-in-4
                # runs, dropping one block's d7/d8 contribution).
                nc.vector.tensor_copy(T78, T78s)
                return (t_b, T2_b, T3_b, T4_b, T5_b, T6_b), T78

            # Prologue DMA order on the Sync queue == rough transfer/land
            # order: first tanh needs xt0's first half, the first matmuls
            # need coeff it=0 degree by degree; everything else follows at
            # the rate the blocks consume it.
            def ct_slice(it, dm1):
                nc.sync.dma_start(ct_tiles[it][:, dm1 * J:(dm1 + 1) * J],
                                  coeff[it][:, dm1 * J:(dm1 + 1) * J])

            def ct_f8(it):
                nc.sync.dma_start(w78_tiles[it], coeff8[it])

            def ct_all(it):
                for dm1 in range(6):
                    ct_slice(it, dm1)
                ct_f8(it)

            def xt_dma(it, half=None):
                if half is None:
                    nc.sync.dma_start(xt_tiles[it],
                                      xt[it * 128:(it + 1) * 128, :])
                else:
                    hs = slice(half * HB, (half + 1) * HB)
                    nc.sync.dma_start(xt_tiles[it][:, hs],
                                      xt[it * 128:(it + 1) * 128, hs])

            xt_dma(0, 0)
            ct_slice(0, 0)
            ct_slice(0, 1)
            xt_dma(1)
            ct_slice(0, 2)
            ct_slice(0, 3)
            xt_dma(2)
            ct_slice(0, 4)
            ct_slice(0, 5)
            ct_f8(0)
            ct_all(1)
            xt_dma(0, 1)
            nc.sync.dma_start(bias_all, bias)
            xt_dma(3)
            ct_all(2)
            xt_dma(4)
            ct_all(3)
            xt_dma(5)
            ct_all(4)
            ct_all(5)

            # Software-pipeline prologue: tanh(0) + chain(0).
            chain_pipe = [None] * NB
            chain_pipe[0] = emit_chain(*issue_tanh(0))

            def drain(jt, ps_t, hs, use_scalar, dma_engine):
                ob = out_pool.tile([128, HB], BF16, name="ob", tag="ob")
                if use_scalar:
                    nc.scalar.activation(ob, ps_t, AF.Identity,
                                         bias=bias_all[:, jt:jt + 1],
                                         scale=INV_S)
                else:
                    obf = out_pool.tile([128, HB], F32, name="obf",
                                        tag="obf")
                    nc.vector.tensor_scalar(obf, ps_t, INV_S, 0.0, OP.mult,
                                            OP.add)
                    nc.vector.tensor_scalar_add(ob, obf,
                                                bias_all[:, jt:jt + 1])
                dma_engine.dma_start(out[jt * 128:(jt + 1) * 128, hs], ob)

            ps = None
            for bi, (half, it) in enumerate(blocks):
                hs = slice(half * HB, (half + 1) * HB)
                ctt = ct_tiles[it]
                w78 = w78_tiles[it]
                if it == 0:
                    ps = [psum_pool.tile([128, HB], F32, name="ps", tag="ps")
                          for _ in range(JT)]
                if bi == 0:
                    # dummy matmuls bridge PE from warm-tile-ready to the
                    # first real matmul; overwritten by the real first matmul
                    # (start=True clears has_written)
                    for _ in range(NDUMMY):
                        nc.tensor.matmul(ps[0], lhsT=warm[:, :128], rhs=warm,
                                         start=True, stop=True)

                # Produce block bi+1's chain now, so every rhs tile exists
                # a block before the PE wants it.
                if bi + 1 < NB:
                    chain_pipe[bi + 1] = emit_chain(
                        *issue_tanh(bi + 1, wait_ms=(12.0 + 9.5 * bi) / 1000))

                Ts, T78 = chain_pipe[bi]
                first = it == 0
                last = it == IT - 1
                if last or bi == IT:
                    # Last block of each pass: jt-major so each j-tile's
                    # accumulation finishes staggered and its PSUM drain
                    # (copy + store) pipelines behind the remaining matmuls;
                    # the next pass finds recycled banks without stalling.
                    for jt in range(JT):
                        for dm1, Td in enumerate(Ts):
                            nc.tensor.matmul(
                                ps[jt],
                                lhsT=ctt[:, dm1 * J + jt * 128:
                                         dm1 * J + (jt + 1) * 128],
                                rhs=Td,
                                start=(first and dm1 == 0),
                                stop=False,
                            )
                        nc.tensor.matmul(
                            ps[jt],
                            lhsT=w78[:, :, jt * 128:(jt + 1) * 128],
                            rhs=T78,
                            perf_mode=DR,
                            start=False,
                            stop=last,
                        )
                        if last:
                            drain(jt, ps[jt], hs, use_scalar=(jt % 2 == 1),
                                  dma_engine=(nc.scalar if jt % 2 == 1
                                              else (nc.sync if half else
                                                    nc.gpsimd)))
                else:
                    for dm1, Td in enumerate(Ts):
                        for jt in range(JT):
                            nc.tensor.matmul(
                                ps[jt],
                                lhsT=ctt[:, dm1 * J + jt * 128:
                                         dm1 * J + (jt + 1) * 128],
                                rhs=Td,
                                start=(first and dm1 == 0),
                                stop=False,
                            )
                    for jt in range(JT):
                        nc.tensor.matmul(
                            ps[jt],
                            lhsT=w78[:, :, jt * 128:(jt + 1) * 128],
                            rhs=T78,
                            perf_mode=DR,
                            start=False,
                            stop=False,
                        )

    nc.compile()
    return nc


def _get_nc():
    if "nc" not in _CACHE:
        _CACHE["nc"] = _build_nc()
    return _CACHE["nc"]


def _prep_inputs(x, coefficients):
    x = np.asarray(x, dtype=np.float32)
    coefficients = np.asarray(coefficients, dtype=np.float32)
    xt_full = np.ascontiguousarray(x.T)  # [768, 8192]

    cr = coefficients.reshape(IT, 128, J, D1) * SCALE
    # coeff[it, row, dm1*J + j] = S * C[it*128+row, j, dm1+1], d = 1..6
    arr6 = np.transpose(cr[:, :, :, 1:7], (0, 1, 3, 2))  # [6, 128, 6, 768]
    coeff_in = np.ascontiguousarray(
        arr6.reshape(IT, 128, 6 * J).astype(ml_dtypes.bfloat16))
    # coeff8[it, row, kd, j] = S * C[it*128+row, j, 7+kd]  (TRN e4m3 caps
    # at +-240; values beyond that are ~impossible for N(0,1) but clip to
    # be safe)
    arr78 = np.transpose(cr[:, :, :, 7:9], (0, 1, 3, 2))  # [6, 128, 2, 768]
    coeff8_in = np.ascontiguousarray(
        np.clip(arr78, -240.0, 240.0).astype(ml_dtypes.float8_e4m3))

    bias_in = np.ascontiguousarray(
        coefficients[:, :, 0].sum(axis=0).astype(np.float32).reshape(JT, 128).T
    )

    in_maps = []
    for c in range(NCORES):
        xt_c = np.ascontiguousarray(xt_full[:, c * BPC:(c + 1) * BPC])
        in_maps.append({"xt": xt_c, "coeff": coeff_in, "coeff8": coeff8_in,
                        "bias": bias_in})
    return in_maps


def _run(x, coefficients, trace=False, **run_kwargs):
    nc = _get_nc()
    in_maps = _prep_inputs(x, coefficients)
    res = bass_utils.run_bass_kernel_spmd(
        nc, in_maps, core_ids=list(range(NCORES)), trace=trace, **run_kwargs
    )
    out_full = np.empty((B, J), dtype=np.float32)
    for c in range(NCORES):
        out_full[c * BPC:(c + 1) * BPC, :] = \
            res.results[c]["out"].T.astype(np.float32)
    return out_full, res


def kernel(x, coefficients):
    out, _ = _run(x, coefficients, trace=False)
    return out


if __name__ == "__main__":
    rng = np.random.default_rng(0)
    x = rng.standard_normal((B, I), dtype=np.float32)
    std = 1.0 / (I * D1)
    coefficients = (std * rng.standard_normal((I, J, D1))).astype(np.float32)
    out = kernel(x, coefficients)
    print("out", out.shape, out.dtype, float(np.abs(out).mean()))
